# revision 43
# baseline (speedup 1.0000x reference)
"""DiscreteWaveletUpsample Trainium2 kernel.

Math: out = conv3x3(haar_upsample(conv3x3(x, pre_w) + pre_b), post_w) + post_b

Device algorithm (per core, one batch sample, data-parallel over batch=8):

  * The fixed Haar reconstruction is folded into the pre-conv weights:
    Y(p,q)[c,h,w] (polyphase components of the upsampled image) is a 3x3
    conv of x with effective weights Weff[p,q,c].

  * x is stored [128, 131, 132] bf16 (host-padded: zero rows/cols on
    every side, so SAME-conv edge handling is free — no wrap-around, no
    repair pass) with partition half 1 holding the image SHIFTED UP one
    row.  Every tap's rhs is a strided 2D window [4 rows x 128 cols].

  * Stage 1 (per 2-tile super): 9 tap-matmuls K=64, 64x128 row-tiled
    with the two streams computing the two p-components CONCURRENTLY
    (stream g does p=g; its shifted half moves the window base up one
    row).  Each w1 tap block is stored once — p=0 content on partition
    rows 0-63, p=1 on 64-127 — so w1 is 295KB instead of 590KB and
    stage 1 starts ~3us earlier.  4 live accumulators + 4 evacuating =
    8 PSUM banks.  Evacuation (ScalarE/VectorE alternating) adds bias,
    writes bf16 Y into zero-bordered [130,130] images: partitions
    q*64+c = Y(p,q)[c].

  * Stage 2 = the post conv in polyphase space, FOUR concurrent streams:
    row group = q_in (the input component's native half -- for every tap
    the two out-components map to distinct q_in, so no partition-swapped
    Y duplicate is needed), col group = tile parity across a pair of
    tiles.  Tap order starts AND stops each accumulator in its own
    quadrant.  p=0 banks evacuate under the p=1 matmuls.

  * Output staged bf16 (halves HBM traffic, 2x DVE rate), host upcast.
"""

import numpy as np
import ml_dtypes

import concourse.bass as bass
import concourse.mybir as mybir
import concourse.tile as tile
from concourse import bacc
from concourse.tile_rust import add_dep_helper
from concourse.bass_utils import run_bass_kernel_spmd

N_CORES = 8

C = 64
H = W = 128
HP = H + 4      # two zero rows above + below (host-side pad image)
HP2 = HP - 1    # on-device rows per half (half 1 is shifted up one row)
WP = W + 4      # two zero cols left + right
# stage-1 tap order (any order valid; center first).
TAPS1 = [(1, 1), (0, 0), (0, 1), (0, 2), (1, 0),
         (1, 2), (2, 0), (2, 1), (2, 2)]
# stage-2 tap order: kx=1 taps first/last so every accumulator's
# start and stop land in its own PE quadrant (q_in == q when kx == 1).
TAPS2 = [(1, 1), (0, 0), (0, 2), (2, 0), (2, 2),
         (1, 0), (1, 2), (0, 1), (2, 1)]

F32 = mybir.dt.float32
BF16 = mybir.dt.bfloat16
NP_BF16 = ml_dtypes.bfloat16
IDENT = mybir.ActivationFunctionType.Identity


# ----------------------------------------------------------------------------
# Host-side weight preparation
# ----------------------------------------------------------------------------

def _build_stage1_weights(pre_w, pre_b):
    """w1n[128, 9*128] bf16-ready blocks + b1[m, p].

    Block ky*3+kx (cols blk*128): rows 0-63 = Weff[p=0, ky, kx], rows
    64-127 = Weff[p=1, ky, kx] — the two row-tiled streams process the
    two p-components concurrently, so each tap block is stored once.
    """
    lo = np.array([0.5, 0.5], np.float32)
    hi = np.array([0.5, -0.5], np.float32)
    filt = np.stack([np.outer(lo, lo), np.outer(lo, hi),
                     np.outer(hi, lo), np.outer(hi, hi)], axis=0)
    pw = pre_w.reshape(4, C, C, 3, 3).astype(np.float32)
    pb = pre_b.reshape(4, C).astype(np.float32)
    weff = np.einsum('spq,scikl->pqcikl', filt, pw)
    beff = np.einsum('spq,sc->pqc', filt, pb)
    w1 = np.transpose(weff, (0, 4, 5, 3, 1, 2)).reshape(2, 3, 3, C, 2 * C)
    b1 = beff.reshape(2, 2 * C).T.copy()
    w1n = np.zeros((128, 9 * 128), np.float32)
    for ky in range(3):
        for kx in range(3):
            blk = (ky * 3 + kx) * 128
            w1n[0:C, blk:blk + 128] = w1[0, ky, kx]
            w1n[C:128, blk:blk + 128] = w1[1, ky, kx]
    return w1n, b1


def _tap_decomp(p, q, ky, kx):
    jy = p + ky - 1
    p_in = jy & 1
    dy = (jy - p_in) >> 1
    jx = q + kx - 1
    q_in = jx & 1
    dx = (jx - q_in) >> 1
    return p_in, q_in, dy, dx


def _build_stage2_weights(post_w):
    """w2[128, 9*64]: block (ky*3+kx) = post_w[:,:,ky,kx].T, both halves."""
    w2 = np.zeros((2 * C, 9 * C), np.float32)
    pwf = post_w.astype(np.float32)
    for ky in range(3):
        for kx in range(3):
            blk = (ky * 3 + kx) * C
            w2[0:C, blk:blk + C] = pwf[:, :, ky, kx].T
            w2[C:2 * C, blk:blk + C] = pwf[:, :, ky, kx].T
    return w2


# ----------------------------------------------------------------------------
# Device module
# ----------------------------------------------------------------------------

def _build_module():
    nc = bacc.Bacc("TRN2", target_bir_lowering=False, debug=False,
                   num_devices=N_CORES)

    # xpad: [C, 132*132] flat; zero rows 0,1,130,131 and zero cols
    # 0,1,130,131 host-baked.
    x_d = nc.dram_tensor("xpad", [C, HP * WP], BF16, kind="ExternalInput")
    w1_d = nc.dram_tensor("w1", [128, 9 * 128], BF16, kind="ExternalInput")
    b1_d = nc.dram_tensor("b1", [128, 2], F32, kind="ExternalInput")
    w2_d = nc.dram_tensor("w2", [128, 9 * C], BF16, kind="ExternalInput")
    b2_d = nc.dram_tensor("b2", [128, 1], F32, kind="ExternalInput")
    out_d = nc.dram_tensor("out", [C, 2 * H, 2 * W], BF16,
                           kind="ExternalOutput")

    with tile.TileContext(nc) as tc:
        with (
            tc.tile_pool(name="const", bufs=1) as const,
            tc.tile_pool(name="xbuf", bufs=1) as xpool,
            tc.tile_pool(name="ybuf", bufs=1) as ypool,
            tc.tile_pool(name="psum", bufs=8, space="PSUM") as psum_pool,
            tc.tile_pool(name="stage", bufs=4) as stg,
        ):
            # ---- x image [131, 132] padded; half 1 holds the image
            # shifted up one row (stream g=1 reads its tap windows one
            # row higher).  8-row mini-chunks up front so stage 1 can
            # start as soon as the first rows land; rows >= 96 ride the
            # scalar queue after w1 (needed only late in stage 1). ----
            xp = xpool.tile([128, HP2, WP], BF16)
            b1_s = const.tile([128, 2], F32)
            w1_s = const.tile([128, 9 * 128], BF16)
            w2_s = const.tile([128, 9 * C], BF16)
            b2_s = const.tile([128, 1], F32)

            def xchunk(g, r0, r1, eng):
                # half g content = xpad rows r+g  (half 1 shifted up)
                eng.dma_start(out=xp[g * C:(g + 1) * C, r0:r1, :],
                              in_=x_d[:, (r0 + g) * WP:(r1 + g) * WP])

            nc.scalar.dma_start(out=b1_s[:], in_=b1_d[:])
            # w1 pieces: center tap (1,1) = block 4 first, then the rest.
            nc.scalar.dma_start(out=w1_s[:, 512:640], in_=w1_d[:, 512:640])
            nc.scalar.dma_start(out=w1_s[:, 0:512], in_=w1_d[:, 0:512])
            nc.scalar.dma_start(out=w1_s[:, 640:], in_=w1_d[:, 640:])
            xchunk(0, 0, 8, nc.sync)
            xchunk(1, 0, 8, nc.gpsimd)
            # dm: dummy rhs/weights for the PE warm-up matmuls.  GpSimd's
            # engine wakes ~1us before Vector's, so memsetting here (right
            # after its critical first x push) starts the PE ramp earlier.
            dm = const.tile([128, 512], BF16)
            nc.gpsimd.memset(dm[:], 0.0)
            for r0, r1 in ((8, 16), (16, 24), (24, 32), (32, 48),
                           (48, 64), (64, 80), (80, 96)):
                xchunk(0, r0, r1, nc.sync)
                xchunk(1, r0, r1, nc.gpsimd)

            # Warm the ScalarE activation table under the input DMAs.
            warm = const.tile([128, 1], F32)
            nc.vector.memset(warm[:], 0.0)
            nc.scalar.activation(warm[:], warm[:], IDENT)

            for g in (0, 1):
                for r0, r1 in ((96, 112), (112, HP2)):
                    xchunk(g, r0, r1, nc.scalar)
            nc.scalar.dma_start(out=w2_s[:], in_=w2_d[:])
            nc.scalar.dma_start(out=b2_s[:], in_=b2_d[:])

            # ---- Y buffers [130,130], zero borders; ybufs[p][j]:
            # partitions 0-63 = Y(p,j), 64-127 = Y(p,1-j); j=0 written by
            # stage-1 evac, j=1 is the partition-swapped DMA copy (so
            # every component is available on both PE row-halves) ----
            ybufs = [[None, None], [None, None]]
            for p in (0, 1):
                for j in (0, 1):
                    yb = ypool.tile([128, H + 2, W + 2], BF16,
                                    name=f"ybuf{p}{j}")
                    ybufs[p][j] = yb
                    nc.gpsimd.memset(yb[:, 0, :], 0.0)
                    nc.gpsimd.memset(yb[:, H + 1, :], 0.0)
                    nc.gpsimd.memset(yb[:, 1:H + 1, 0:1], 0.0)
                    nc.gpsimd.memset(yb[:, 1:H + 1, W + 1:W + 2], 0.0)

            # ---- global PE emission-order chain ----
            state = {"prev": None}

            def mm(out_ap, w_ap, rhs_ap, start, stop, pos):
                inst = nc.tensor.matmul(out_ap, w_ap, rhs_ap,
                                        start=start, stop=stop,
                                        tile_position=pos)
                if state["prev"] is not None:
                    add_dep_helper(inst.ins, state["prev"], sync=False,
                                   reason="pe-emission-order")
                state["prev"] = inst.ins

            # ---- PE warm-up: dummy matmuls keep the tensor engine busy
            # while the x/w1 DMAs land (stage 1 is data-gated until
            # ~13us) so the clock ramps to the high p-state before the
            # real stream starts ----
            for i in range(31):
                g = i % 2
                acc = psum_pool.tile([128, 4, W], F32, name="ps", tag="ps")
                mm(acc[:, :, :], dm[g * C:(g + 1) * C, 0:128],
                   dm[g * C:(g + 1) * C, :], True, True, (g * C, 0))

            def stage1_super(sup):
                # 2-tile supers: 4 live accumulators (p x t) + 4
                # evacuating = 8 PSUM banks.  Row-tiled stream g computes
                # component p=g (block rows g*64..), its shifted x half
                # moving the window base up one row.
                ts_all = [2 * sup, 2 * sup + 1]
                accs = {(p, t): psum_pool.tile([128, 4, W], F32,
                                               name="ps", tag="ps")
                        for p in (0, 1) for t in ts_all}
                for k, (ky, kx) in enumerate(TAPS1):
                    blk = (ky * 3 + kx) * 128
                    for g in (0, 1):
                        gs = slice(g * C, (g + 1) * C)
                        for t in ts_all:
                            r0 = 4 * t + ky + 1 - g
                            mm(accs[g, t][:, :, :],
                               w1_s[gs, blk:blk + 128],
                               xp[gs, r0:r0 + 4, kx + 1:kx + 1 + W],
                               k == 0, k == 8, (g * C, 0))
                for p in (0, 1):
                    for t in ts_all:
                        h0 = 4 * t
                        dst = ybufs[p][0][:, h0 + 1:h0 + 5, 1:W + 1]
                        if (p + t) % 2 == 0:
                            nc.scalar.activation(dst, accs[p, t][:, :, :],
                                                 IDENT,
                                                 bias=b1_s[:, p:p + 1])
                        else:
                            nc.vector.tensor_scalar_add(
                                dst, accs[p, t][:, :, :], b1_s[:, p:p + 1])
                    # partition-swapped duplicate, one chunk per (p, half)
                    r0, r1 = 4 * ts_all[0] + 1, 4 * ts_all[-1] + 5
                    nc.sync.dma_start(
                        out=ybufs[p][1][0:C, r0:r1, :],
                        in_=ybufs[p][0][C:128, r0:r1, :])
                    nc.gpsimd.dma_start(
                        out=ybufs[p][1][C:128, r0:r1, :],
                        in_=ybufs[p][0][0:C, r0:r1, :])

            def stage2_pair(j):
                st = stg.tile([128, 8, 2 * W], BF16, name="st", tag="st")
                # One bank per (pp, qq): the c=0/c=1 col-group streams
                # write DISJOINT partition halves of the same bank
                # (per-partition PSUM SRAMs — no RMW race), so a pair
                # needs 4 banks and HALF the evacuation ops.
                accs = {}
                for pp in (0, 1):
                    for qq in (0, 1):
                        accs[pp, qq] = psum_pool.tile(
                            [128, 4, W], F32, name="ps", tag="ps")
                for pp in (0, 1):
                    for k, (ky, kx) in enumerate(TAPS2):
                        blk = (ky * 3 + kx) * C
                        for qq in (0, 1):
                            for c in (0, 1):
                                h0 = 4 * (2 * j + c)
                                cs = slice(c * C, (c + 1) * C)
                                # fixed row group g=qq per accumulator
                                # region (one quadrant per partition
                                # range: quadrants run concurrently).
                                p_in, q_in, dy, dx = _tap_decomp(
                                    pp, qq, ky, kx)
                                gs = slice(qq * C, (qq + 1) * C)
                                rhs = ybufs[p_in][q_in ^ qq][
                                    gs, h0 + dy + 1:h0 + dy + 5,
                                    dx + 1:dx + 1 + W]
                                mm(accs[pp, qq][cs, :, :],
                                   w2_s[gs, blk:blk + C],
                                   rhs, k == 0, k == 8, (qq * C, c * C))
                    for qq in (0, 1):
                        dst = st[:, pp::2, qq::2]
                        src = accs[pp, qq][:, :, :]
                        if (qq + pp) % 2 == 0:
                            nc.scalar.activation(dst, src, IDENT,
                                                 bias=b2_s[:, 0:1])
                        else:
                            nc.vector.tensor_scalar_add(
                                dst, src, b2_s[:, 0:1])
                    if j == H // 8 - 1:
                        # final pair: write each pp's rows as soon as its
                        # evacs land (even rows overlap the pp=1 matmuls)
                        engs = ((nc.sync, nc.gpsimd) if pp == 0
                                else (nc.scalar, nc.sync))
                        for c, eng in zip((0, 1), engs):
                            t = 2 * j + c
                            cs = slice(c * C, (c + 1) * C)
                            eng.dma_start(
                                out=out_d[:, 8 * t + pp:8 * t + 8:2, :],
                                in_=st[cs, pp::2, :])
                if j < H // 8 - 2:
                    for c, eng in ((0, nc.sync), (1, nc.gpsimd)):
                        t = 2 * j + c
                        cs = slice(c * C, (c + 1) * C)
                        eng.dma_start(out=out_d[:, 8 * t:8 * t + 8, :],
                                      in_=st[cs, :, :])
                elif j == H // 8 - 2:
                    # 4-row chunks, two rings, keeping the scalar/vector
                    # engines free for the final evacuations.
                    for c, h, eng in ((0, 0, nc.sync), (0, 4, nc.gpsimd),
                                      (1, 0, nc.scalar), (1, 4, nc.sync)):
                        t = 2 * j + c
                        cs = slice(c * C, (c + 1) * C)
                        eng.dma_start(
                            out=out_d[:, 8 * t + h:8 * t + h + 4, :],
                            in_=st[cs, h:h + 4, :])

            for sup in range(H // 8):
                stage1_super(sup)
            for j in range(H // 8):
                stage2_pair(j)

    nc.compile()
    return nc


_MODULE_CACHE = {}


def _get_module():
    if "nc" not in _MODULE_CACHE:
        _MODULE_CACHE["nc"] = _build_module()
    return _MODULE_CACHE["nc"]


# ----------------------------------------------------------------------------
# Entry point
# ----------------------------------------------------------------------------

def prep_weight_map(pre_w, pre_b, post_w, post_b):
    w1n, b1 = _build_stage1_weights(np.asarray(pre_w), np.asarray(pre_b))
    w2 = _build_stage2_weights(np.asarray(post_w))
    b2 = np.asarray(post_b, np.float32).reshape(C, 1)

    return {
        "w1": np.ascontiguousarray(w1n).astype(NP_BF16),
        "b1": np.ascontiguousarray(b1, np.float32),
        "w2": np.ascontiguousarray(w2).astype(NP_BF16),
        "b2": np.ascontiguousarray(np.vstack([b2, b2]), np.float32),
    }


def run(x, pre_w, pre_b, post_w, post_b, trace=False):
    x = np.asarray(x, np.float32)
    B = x.shape[0]
    assert B == N_CORES and x.shape == (B, C, H, W)

    wmap = prep_weight_map(pre_w, pre_b, post_w, post_b)
    x_bf = x.astype(NP_BF16)

    in_maps = []
    for b in range(B):
        xpad = np.zeros((C, HP, WP), NP_BF16)
        xpad[:, 2:H + 2, 2:W + 2] = x_bf[b]
        in_maps.append({
            "xpad": np.ascontiguousarray(xpad.reshape(C, HP * WP)),
            **wmap,
        })

    nc = _get_module()
    res = run_bass_kernel_spmd(nc, in_maps, core_ids=list(range(N_CORES)),
                               trace=trace)
    out = np.stack([res.results[b]["out"].astype(np.float32)
                    for b in range(B)])
    return out, res


def kernel(x, pre_w, pre_b, post_w, post_b):
    out, _ = run(x, pre_w, pre_b, post_w, post_b)
    return out


# revision 44
# speedup vs baseline: 1.0018x; 1.0018x over previous
"""DiscreteWaveletUpsample Trainium2 kernel.

Math: out = conv3x3(haar_upsample(conv3x3(x, pre_w) + pre_b), post_w) + post_b

Device algorithm (per core, one batch sample, data-parallel over batch=8):

  * The fixed Haar reconstruction is folded into the pre-conv weights:
    Y(p,q)[c,h,w] (polyphase components of the upsampled image) is a 3x3
    conv of x with effective weights Weff[p,q,c].

  * x is stored [128, 131, 132] bf16 (host-padded: zero rows/cols on
    every side, so SAME-conv edge handling is free — no wrap-around, no
    repair pass) with partition half 1 holding the image SHIFTED UP one
    row.  Every tap's rhs is a strided 2D window [4 rows x 128 cols].

  * Stage 1 (per 2-tile super): 9 tap-matmuls K=64, 64x128 row-tiled
    with the two streams computing the two p-components CONCURRENTLY
    (stream g does p=g; its shifted half moves the window base up one
    row).  Each w1 tap block is stored once — p=0 content on partition
    rows 0-63, p=1 on 64-127 — so w1 is 295KB instead of 590KB and
    stage 1 starts ~3us earlier.  4 live accumulators + 4 evacuating =
    8 PSUM banks.  Evacuation (ScalarE/VectorE alternating) adds bias,
    writes bf16 Y into zero-bordered [130,130] images: partitions
    q*64+c = Y(p,q)[c].

  * Stage 2 = the post conv in polyphase space, FOUR concurrent streams:
    row group = q_in (the input component's native half -- for every tap
    the two out-components map to distinct q_in, so no partition-swapped
    Y duplicate is needed), col group = tile parity across a pair of
    tiles.  Tap order starts AND stops each accumulator in its own
    quadrant.  p=0 banks evacuate under the p=1 matmuls.

  * Output staged bf16 (halves HBM traffic, 2x DVE rate), host upcast.
"""

import numpy as np
import ml_dtypes

import concourse.bass as bass
import concourse.mybir as mybir
import concourse.tile as tile
from concourse import bacc
from concourse.tile_rust import add_dep_helper
from concourse.bass_utils import run_bass_kernel_spmd

N_CORES = 8

C = 64
H = W = 128
HP = H + 4      # two zero rows above + below (host-side pad image)
HP2 = HP - 1    # on-device rows per half (half 1 is shifted up one row)
WP = W + 4      # two zero cols left + right
# stage-1 tap order (any order valid; center first).
TAPS1 = [(1, 1), (0, 0), (0, 1), (0, 2), (1, 0),
         (1, 2), (2, 0), (2, 1), (2, 2)]
# stage-2 tap order: kx=1 taps first/last so every accumulator's
# start and stop land in its own PE quadrant (q_in == q when kx == 1).
TAPS2 = [(1, 1), (0, 0), (0, 2), (2, 0), (2, 2),
         (1, 0), (1, 2), (0, 1), (2, 1)]

F32 = mybir.dt.float32
BF16 = mybir.dt.bfloat16
NP_BF16 = ml_dtypes.bfloat16
IDENT = mybir.ActivationFunctionType.Identity


# ----------------------------------------------------------------------------
# Host-side weight preparation
# ----------------------------------------------------------------------------

def _build_stage1_weights(pre_w, pre_b):
    """w1n[128, 9*128] bf16-ready blocks + b1[m, p].

    Block ky*3+kx (cols blk*128): rows 0-63 = Weff[p=0, ky, kx], rows
    64-127 = Weff[p=1, ky, kx] — the two row-tiled streams process the
    two p-components concurrently, so each tap block is stored once.
    """
    lo = np.array([0.5, 0.5], np.float32)
    hi = np.array([0.5, -0.5], np.float32)
    filt = np.stack([np.outer(lo, lo), np.outer(lo, hi),
                     np.outer(hi, lo), np.outer(hi, hi)], axis=0)
    pw = pre_w.reshape(4, C, C, 3, 3).astype(np.float32)
    pb = pre_b.reshape(4, C).astype(np.float32)
    weff = np.einsum('spq,scikl->pqcikl', filt, pw)
    beff = np.einsum('spq,sc->pqc', filt, pb)
    w1 = np.transpose(weff, (0, 4, 5, 3, 1, 2)).reshape(2, 3, 3, C, 2 * C)
    b1 = beff.reshape(2, 2 * C).T.copy()
    w1n = np.zeros((128, 9 * 128), np.float32)
    for ky in range(3):
        for kx in range(3):
            blk = (ky * 3 + kx) * 128
            w1n[0:C, blk:blk + 128] = w1[0, ky, kx]
            w1n[C:128, blk:blk + 128] = w1[1, ky, kx]
    return w1n, b1


def _tap_decomp(p, q, ky, kx):
    jy = p + ky - 1
    p_in = jy & 1
    dy = (jy - p_in) >> 1
    jx = q + kx - 1
    q_in = jx & 1
    dx = (jx - q_in) >> 1
    return p_in, q_in, dy, dx


def _build_stage2_weights(post_w):
    """w2[128, 9*64]: block (ky*3+kx) = post_w[:,:,ky,kx].T, both halves."""
    w2 = np.zeros((2 * C, 9 * C), np.float32)
    pwf = post_w.astype(np.float32)
    for ky in range(3):
        for kx in range(3):
            blk = (ky * 3 + kx) * C
            w2[0:C, blk:blk + C] = pwf[:, :, ky, kx].T
            w2[C:2 * C, blk:blk + C] = pwf[:, :, ky, kx].T
    return w2


# ----------------------------------------------------------------------------
# Device module
# ----------------------------------------------------------------------------

def _build_module():
    nc = bacc.Bacc("TRN2", target_bir_lowering=False, debug=False,
                   num_devices=N_CORES)

    # xpad: [C, 132*132] flat; zero rows 0,1,130,131 and zero cols
    # 0,1,130,131 host-baked.
    x_d = nc.dram_tensor("xpad", [C, HP * WP], BF16, kind="ExternalInput")
    w1_d = nc.dram_tensor("w1", [128, 9 * 128], BF16, kind="ExternalInput")
    b1_d = nc.dram_tensor("b1", [128, 2], F32, kind="ExternalInput")
    w2_d = nc.dram_tensor("w2", [128, 9 * C], BF16, kind="ExternalInput")
    b2_d = nc.dram_tensor("b2", [128, 1], F32, kind="ExternalInput")
    out_d = nc.dram_tensor("out", [C, 2 * H, 2 * W], BF16,
                           kind="ExternalOutput")

    with tile.TileContext(nc) as tc:
        with (
            tc.tile_pool(name="const", bufs=1) as const,
            tc.tile_pool(name="xbuf", bufs=1) as xpool,
            tc.tile_pool(name="ybuf", bufs=1) as ypool,
            tc.tile_pool(name="psum", bufs=8, space="PSUM") as psum_pool,
            tc.tile_pool(name="stage", bufs=4) as stg,
        ):
            # ---- x image [131, 132] padded; half 1 holds the image
            # shifted up one row (stream g=1 reads its tap windows one
            # row higher).  8-row mini-chunks up front so stage 1 can
            # start as soon as the first rows land; rows >= 96 ride the
            # scalar queue after w1 (needed only late in stage 1). ----
            xp = xpool.tile([128, HP2, WP], BF16)
            b1_s = const.tile([128, 2], F32)
            w1_s = const.tile([128, 9 * 128], BF16)
            w2_s = const.tile([128, 9 * C], BF16)
            b2_s = const.tile([128, 1], F32)

            def xchunk(g, r0, r1, eng):
                # half g content = xpad rows r+g  (half 1 shifted up)
                eng.dma_start(out=xp[g * C:(g + 1) * C, r0:r1, :],
                              in_=x_d[:, (r0 + g) * WP:(r1 + g) * WP])

            nc.scalar.dma_start(out=b1_s[:], in_=b1_d[:])
            # w1 pieces: center tap (1,1) = block 4 first, then the rest.
            nc.scalar.dma_start(out=w1_s[:, 512:640], in_=w1_d[:, 512:640])
            nc.scalar.dma_start(out=w1_s[:, 0:512], in_=w1_d[:, 0:512])
            nc.scalar.dma_start(out=w1_s[:, 640:], in_=w1_d[:, 640:])
            xchunk(0, 0, 8, nc.sync)
            xchunk(1, 0, 8, nc.gpsimd)
            # dm: dummy rhs/weights for the PE warm-up matmuls.  GpSimd's
            # engine wakes ~1us before Vector's, so memsetting here (right
            # after its critical first x push) starts the PE ramp earlier.
            dm = const.tile([128, 512], BF16)
            nc.gpsimd.memset(dm[:], 0.0)
            for r0, r1 in ((8, 16), (16, 24), (24, 32), (32, 48),
                           (48, 64), (64, 80), (80, 96)):
                xchunk(0, r0, r1, nc.sync)
                xchunk(1, r0, r1, nc.gpsimd)

            # Warm the ScalarE activation table under the input DMAs.
            warm = const.tile([128, 1], F32)
            nc.vector.memset(warm[:], 0.0)
            nc.scalar.activation(warm[:], warm[:], IDENT)

            for g in (0, 1):
                for r0, r1 in ((96, 112), (112, HP2)):
                    xchunk(g, r0, r1, nc.scalar)
            nc.scalar.dma_start(out=w2_s[:], in_=w2_d[:])
            nc.scalar.dma_start(out=b2_s[:], in_=b2_d[:])

            # ---- Y buffers [130,130], zero borders; ybufs[p][j]:
            # partitions 0-63 = Y(p,j), 64-127 = Y(p,1-j); j=0 written by
            # stage-1 evac, j=1 is the partition-swapped DMA copy (so
            # every component is available on both PE row-halves) ----
            ybufs = [[None, None], [None, None]]
            for p in (0, 1):
                for j in (0, 1):
                    yb = ypool.tile([128, H + 2, W + 2], BF16,
                                    name=f"ybuf{p}{j}")
                    ybufs[p][j] = yb
                    nc.gpsimd.memset(yb[:, 0, :], 0.0)
                    nc.gpsimd.memset(yb[:, H + 1, :], 0.0)
                    nc.gpsimd.memset(yb[:, 1:H + 1, 0:1], 0.0)
                    nc.gpsimd.memset(yb[:, 1:H + 1, W + 1:W + 2], 0.0)

            # ---- global PE emission-order chain ----
            state = {"prev": None}

            def mm(out_ap, w_ap, rhs_ap, start, stop, pos):
                inst = nc.tensor.matmul(out_ap, w_ap, rhs_ap,
                                        start=start, stop=stop,
                                        tile_position=pos)
                if state["prev"] is not None:
                    add_dep_helper(inst.ins, state["prev"], sync=False,
                                   reason="pe-emission-order")
                state["prev"] = inst.ins

            # ---- PE warm-up: dummy matmuls keep the tensor engine busy
            # while the x/w1 DMAs land (stage 1 is data-gated until
            # ~13us) so the clock ramps to the high p-state before the
            # real stream starts ----
            for i in range(31):
                g = i % 2
                acc = psum_pool.tile([128, 4, W], F32, name="ps", tag="ps")
                mm(acc[:, :, :], dm[g * C:(g + 1) * C, 0:128],
                   dm[g * C:(g + 1) * C, :], True, True, (g * C, 0))

            def stage1_super(sup):
                # 2-tile supers: 4 live accumulators (p x t) + 4
                # evacuating = 8 PSUM banks.  Row-tiled stream g computes
                # component p=g (block rows g*64..), its shifted x half
                # moving the window base up one row.
                ts_all = [2 * sup, 2 * sup + 1]
                accs = {(p, t): psum_pool.tile([128, 4, W], F32,
                                               name="ps", tag="ps")
                        for p in (0, 1) for t in ts_all}
                for k, (ky, kx) in enumerate(TAPS1):
                    blk = (ky * 3 + kx) * 128
                    for g in (0, 1):
                        gs = slice(g * C, (g + 1) * C)
                        for t in ts_all:
                            r0 = 4 * t + ky + 1 - g
                            mm(accs[g, t][:, :, :],
                               w1_s[gs, blk:blk + 128],
                               xp[gs, r0:r0 + 4, kx + 1:kx + 1 + W],
                               k == 0, k == 8, (g * C, 0))
                for p in (0, 1):
                    for t in ts_all:
                        h0 = 4 * t
                        dst = ybufs[p][0][:, h0 + 1:h0 + 5, 1:W + 1]
                        if (p + t) % 2 == 0:
                            nc.scalar.activation(dst, accs[p, t][:, :, :],
                                                 IDENT,
                                                 bias=b1_s[:, p:p + 1])
                        else:
                            nc.vector.tensor_scalar_add(
                                dst, accs[p, t][:, :, :], b1_s[:, p:p + 1])
                    # partition-swapped duplicate, one chunk per (p, half)
                    r0, r1 = 4 * ts_all[0] + 1, 4 * ts_all[-1] + 5
                    nc.sync.dma_start(
                        out=ybufs[p][1][0:C, r0:r1, :],
                        in_=ybufs[p][0][C:128, r0:r1, :])
                    nc.gpsimd.dma_start(
                        out=ybufs[p][1][C:128, r0:r1, :],
                        in_=ybufs[p][0][0:C, r0:r1, :])

            def stage2_pair(j):
                st = stg.tile([128, 8, 2 * W], BF16, name="st", tag="st")
                # One bank per (pp, qq): the c=0/c=1 col-group streams
                # write DISJOINT partition halves of the same bank
                # (per-partition PSUM SRAMs — no RMW race), so a pair
                # needs 4 banks and HALF the evacuation ops.
                accs = {}
                for pp in (0, 1):
                    for qq in (0, 1):
                        accs[pp, qq] = psum_pool.tile(
                            [128, 4, W], F32, name="ps", tag="ps")
                for pp in (0, 1):
                    for k, (ky, kx) in enumerate(TAPS2):
                        blk = (ky * 3 + kx) * C
                        for c in (0, 1):
                            h0 = 4 * (2 * j + c)
                            cs = slice(c * C, (c + 1) * C)
                            for qq in (0, 1):
                                # fixed row group g=qq per accumulator
                                # region (one quadrant per partition
                                # range: quadrants run concurrently).
                                p_in, q_in, dy, dx = _tap_decomp(
                                    pp, qq, ky, kx)
                                gs = slice(qq * C, (qq + 1) * C)
                                rhs = ybufs[p_in][q_in ^ qq][
                                    gs, h0 + dy + 1:h0 + dy + 5,
                                    dx + 1:dx + 1 + W]
                                mm(accs[pp, qq][cs, :, :],
                                   w2_s[gs, blk:blk + C],
                                   rhs, k == 0, k == 8, (qq * C, c * C))
                    for qq in (0, 1):
                        dst = st[:, pp::2, qq::2]
                        src = accs[pp, qq][:, :, :]
                        if (qq + pp) % 2 == 0:
                            nc.scalar.activation(dst, src, IDENT,
                                                 bias=b2_s[:, 0:1])
                        else:
                            nc.vector.tensor_scalar_add(
                                dst, src, b2_s[:, 0:1])
                    if j == H // 8 - 1:
                        # final pair: write each pp's rows as soon as its
                        # evacs land (even rows overlap the pp=1 matmuls)
                        engs = ((nc.sync, nc.gpsimd) if pp == 0
                                else (nc.scalar, nc.sync))
                        for c, eng in zip((0, 1), engs):
                            t = 2 * j + c
                            cs = slice(c * C, (c + 1) * C)
                            eng.dma_start(
                                out=out_d[:, 8 * t + pp:8 * t + 8:2, :],
                                in_=st[cs, pp::2, :])
                if j < H // 8 - 2:
                    for c, eng in ((0, nc.sync), (1, nc.gpsimd)):
                        t = 2 * j + c
                        cs = slice(c * C, (c + 1) * C)
                        eng.dma_start(out=out_d[:, 8 * t:8 * t + 8, :],
                                      in_=st[cs, :, :])
                elif j == H // 8 - 2:
                    # 4-row chunks, two rings, keeping the scalar/vector
                    # engines free for the final evacuations.
                    for c, h, eng in ((0, 0, nc.sync), (0, 4, nc.gpsimd),
                                      (1, 0, nc.scalar), (1, 4, nc.sync)):
                        t = 2 * j + c
                        cs = slice(c * C, (c + 1) * C)
                        eng.dma_start(
                            out=out_d[:, 8 * t + h:8 * t + h + 4, :],
                            in_=st[cs, h:h + 4, :])

            for sup in range(H // 8):
                stage1_super(sup)
            for j in range(H // 8):
                stage2_pair(j)

    nc.compile()
    return nc


_MODULE_CACHE = {}


def _get_module():
    if "nc" not in _MODULE_CACHE:
        _MODULE_CACHE["nc"] = _build_module()
    return _MODULE_CACHE["nc"]


# ----------------------------------------------------------------------------
# Entry point
# ----------------------------------------------------------------------------

def prep_weight_map(pre_w, pre_b, post_w, post_b):
    w1n, b1 = _build_stage1_weights(np.asarray(pre_w), np.asarray(pre_b))
    w2 = _build_stage2_weights(np.asarray(post_w))
    b2 = np.asarray(post_b, np.float32).reshape(C, 1)

    return {
        "w1": np.ascontiguousarray(w1n).astype(NP_BF16),
        "b1": np.ascontiguousarray(b1, np.float32),
        "w2": np.ascontiguousarray(w2).astype(NP_BF16),
        "b2": np.ascontiguousarray(np.vstack([b2, b2]), np.float32),
    }


def run(x, pre_w, pre_b, post_w, post_b, trace=False):
    x = np.asarray(x, np.float32)
    B = x.shape[0]
    assert B == N_CORES and x.shape == (B, C, H, W)

    wmap = prep_weight_map(pre_w, pre_b, post_w, post_b)
    x_bf = x.astype(NP_BF16)

    in_maps = []
    for b in range(B):
        xpad = np.zeros((C, HP, WP), NP_BF16)
        xpad[:, 2:H + 2, 2:W + 2] = x_bf[b]
        in_maps.append({
            "xpad": np.ascontiguousarray(xpad.reshape(C, HP * WP)),
            **wmap,
        })

    nc = _get_module()
    res = run_bass_kernel_spmd(nc, in_maps, core_ids=list(range(N_CORES)),
                               trace=trace)
    out = np.stack([res.results[b]["out"].astype(np.float32)
                    for b in range(B)])
    return out, res


def kernel(x, pre_w, pre_b, post_w, post_b):
    out, _ = run(x, pre_w, pre_b, post_w, post_b)
    return out
